# revision 2
# baseline (speedup 1.0000x reference)
"""Causal self-attention kernel for 8 Trainium2 NeuronCores.

Problem: B=4, T=2048, C=1024, H=16 heads, HD=64.
  qkv = hs @ qkv_w.T + qkv_b ; per-head causal softmax attention ;
  out = attn @ o_w.T + o_b

Sharding (8 cores): core c handles batch b = c//2 and head-half g = c%2
(8 heads). Each core computes q/k/v for its heads from its batch's
hidden states, runs causal attention, and produces a partial output
projection over its 512 attention-output channels. The host sums the
two partials per batch and adds o_b.

On-device layout/dataflow (per core):
  - host pre-transposes hs and weights so every matmul contraction dim
    lands on SBUF partitions with contiguous DMA lines (no on-device
    transposes).
  - qT, kT stored [d, t] (d on partitions); v stored [t, d] natural,
    augmented with a ones-column so the PV matmul's row 64 accumulates
    the softmax denominator for free.
  - scores computed transposed [j, q] in PSUM; softmax has no
    max-subtraction (scores are ~N(0,1); exp is safe in fp32);
    causal mask applied multiplicatively on the exp'd tile only for
    diagonal-straddling j-chunks.
  - two heads share the PE array via tile_position row packing (K=64).
  - all matmuls in float32r (full PE rate, ~1.6e-4 relative rounding).
"""
import numpy as np
from contextlib import ExitStack

import concourse.bass as bass
from concourse import bacc
import concourse.tile as tile
import concourse.mybir as mybir
from concourse.bass_utils import run_bass_kernel_spmd

B, T, C = 4, 2048, 1024
H, HD = 16, 64
NCORES = 8
HPC = H // 2            # 8 heads per core
E = HPC * HD            # 512 local attn-out channels per core
P = 128
SC = 512                # q-chunk (matmul free dim)
NQC = T // SC           # 4 q-chunks
NJC = T // P            # 16 j-chunks
F32 = mybir.dt.float32
F32R = mybir.dt.float32r
Exp = mybir.ActivationFunctionType.Exp
SCALE = HD ** -0.5

_cache = {}


def _build():
    nc = bacc.Bacc("TRN2", target_bir_lowering=False, debug=False)
    hsT = nc.dram_tensor("hsT", [C, T], F32R, kind="ExternalInput")
    wqkvT = nc.dram_tensor("wqkvT", [C, 3 * E], F32R, kind="ExternalInput")
    woT = nc.dram_tensor("woT", [E, C], F32R, kind="ExternalInput")
    bqkv = nc.dram_tensor("bqkv", [P, 8], F32, kind="ExternalInput")
    vbias = nc.dram_tensor("vbias", [P, E], F32, kind="ExternalInput")
    masks = nc.dram_tensor("masks", [P, 4, SC], F32, kind="ExternalInput")
    outp = nc.dram_tensor("outp", [T, C], F32, kind="ExternalOutput")

    with tile.TileContext(nc) as tc, ExitStack() as ctx:
        const_pool = ctx.enter_context(tc.tile_pool(name="const", bufs=1))
        qk_pool = ctx.enter_context(tc.tile_pool(name="qk", bufs=1))

        bqkv_sb = const_pool.tile([P, 8], F32)
        vbias_sb = const_pool.tile([P, E], F32)
        masks_sb = const_pool.tile([P, 4, SC], F32)
        ones_sb = const_pool.tile([P, 1], F32)
        nc.sync.dma_start(bqkv_sb[:], bqkv.ap())
        nc.sync.dma_start(vbias_sb[:], vbias.ap())
        nc.sync.dma_start(masks_sb[:], masks.ap())
        nc.vector.memset(ones_sb[:], 1.0)

        qT = qk_pool.tile([P, 4, T], F32R)            # [d%128, d//128, t]
        kT = qk_pool.tile([P, 4, T], F32R)
        v_aug = qk_pool.tile([P, NJC, HPC, HD + 1], F32R)  # [t%128, jc, h, d|1]
        nc.vector.tensor_copy(
            v_aug[:, :, :, HD], ones_sb[:, 0, None, None].to_broadcast((P, NJC, HPC))
        )

        # ---- Phase 1: fused QKV projection ----
        with tc.tile_pool(name="wq", bufs=1) as wq_pool, \
             tc.tile_pool(name="hst", bufs=2) as hst_pool, \
             tc.tile_pool(name="psq", bufs=4, space="PSUM") as psq:
            wqkvT_sb = wq_pool.tile([P, C // P, 3 * E], F32R)
            nc.sync.dma_start(
                wqkvT_sb[:], wqkvT.ap().rearrange("(co p) d -> p co d", p=P)
            )
            for t4 in range(NQC):
                hst = hst_pool.tile([P, C // P, SC], F32R, tag="hst")
                nc.sync.dma_start(
                    hst[:],
                    hsT.ap()[:, t4 * SC:(t4 + 1) * SC].rearrange(
                        "(co p) t -> p co t", p=P
                    ),
                )
                for qk in range(2):  # 0 -> qT, 1 -> kT
                    dstT = qT if qk == 0 else kT
                    for dc in range(4):
                        ps = psq.tile([P, SC], F32, tag="psq")
                        w0 = qk * E + dc * P
                        for cc in range(C // P):
                            nc.tensor.matmul(
                                ps[:], wqkvT_sb[:, cc, w0:w0 + P], hst[:, cc, :],
                                start=(cc == 0), stop=(cc == C // P - 1),
                            )
                        nc.vector.tensor_add(
                            dstT[:, dc, t4 * SC:(t4 + 1) * SC], ps[:],
                            bqkv_sb[:, qk * 4 + dc, None].to_broadcast((P, SC)),
                        )
                for ts in range(4):  # v natural [t, d]
                    ps = psq.tile([P, E], F32, tag="psq")
                    for cc in range(C // P):
                        nc.tensor.matmul(
                            ps[:], hst[:, cc, ts * P:(ts + 1) * P],
                            wqkvT_sb[:, cc, 2 * E:3 * E],
                            start=(cc == 0), stop=(cc == C // P - 1),
                        )
                    jc = t4 * 4 + ts
                    nc.vector.tensor_add(
                        v_aug[:, jc, :, 0:HD],
                        ps[:].rearrange("p (h d) -> p h d", d=HD),
                        vbias_sb[:].rearrange("p (h d) -> p h d", d=HD),
                    )

        # ---- Phase 2: attention + output projection ----
        wo_pool = ctx.enter_context(tc.tile_pool(name="wo", bufs=1))
        attnp_pool = ctx.enter_context(tc.tile_pool(name="attnp", bufs=2))
        exp_pool = ctx.enter_context(tc.tile_pool(name="expp", bufs=6))
        bc_pool = ctx.enter_context(tc.tile_pool(name="bcp", bufs=4))
        rc_pool = ctx.enter_context(tc.tile_pool(name="rcp", bufs=4))
        ost_pool = ctx.enter_context(tc.tile_pool(name="ost", bufs=3))
        ps_sc = ctx.enter_context(tc.tile_pool(name="ps_sc", bufs=4, space="PSUM"))
        ps_out = ctx.enter_context(tc.tile_pool(name="ps_out", bufs=2, space="PSUM"))
        ps_op = ctx.enter_context(tc.tile_pool(name="ps_op", bufs=2, space="PSUM"))

        woT_sb = wo_pool.tile([P, E // P, C], F32R)
        nc.sync.dma_start(woT_sb[:], woT.ap().rearrange("(ec p) co -> p ec co", p=P))

        for qc in range(NQC):
            attnp = attnp_pool.tile([P, E // P, SC], F32R, tag="attnp")
            q0 = qc * SC
            nj = 4 * (qc + 1)
            for hp in range(4):  # head pairs (2hp, 2hp+1)
                out_ps = [
                    ps_out.tile([HD + 1, SC], F32, tag="outps", name=f"outps{s}")
                    for s in range(2)
                ]
                pend = None  # deferred PV pair (software pipeline skew)

                def emit_pv(item):
                    jc, n0, exps = item
                    for s in range(2):
                        nc.tensor.matmul(
                            out_ps[s][:, n0:SC], v_aug[:, jc, 2 * hp + s, :],
                            exps[s][:, n0:SC],
                            start=(jc == 0), stop=(jc == nj - 1),
                        )

                for jc in range(nj):
                    di = jc - 4 * qc  # >= 0 on diagonal-straddling chunks
                    n0 = P * di if di >= 0 else 0
                    j0 = jc * P
                    scs, exps = [], []
                    for s in range(2):
                        sc_ps = ps_sc.tile([P, SC], F32, tag="scps")
                        nc.tensor.matmul(
                            sc_ps[:, n0:SC],
                            kT[64 * s:64 * s + 64, hp, j0:j0 + P],
                            qT[64 * s:64 * s + 64, hp, q0 + n0:q0 + SC],
                            start=True, stop=True, tile_position=(64 * s, 0),
                        )
                        scs.append(sc_ps)
                    if pend is not None:
                        emit_pv(pend)
                    for s in range(2):
                        e = exp_pool.tile([P, SC], F32R, tag="exp")
                        nc.scalar.activation(
                            e[:, n0:SC], scs[s][:, n0:SC], Exp, scale=SCALE
                        )
                        if di >= 0:
                            nc.vector.tensor_mul(
                                e[:, n0:SC], e[:, n0:SC].bitcast(F32),
                                masks_sb[:, di, n0:SC],
                            )
                        exps.append(e)
                    pend = (jc, n0, exps)
                emit_pv(pend)
                # normalize by the ones-row sum and place into attnp
                for s in range(2):
                    rc = rc_pool.tile([1, SC], F32, tag="rc")
                    nc.vector.reciprocal(rc[:], out_ps[s][HD:HD + 1, :])
                    bc = bc_pool.tile([64, SC], F32, tag="bc")
                    nc.gpsimd.partition_broadcast(bc[:], rc[:])
                    nc.vector.tensor_mul(
                        attnp[64 * s:64 * s + 64, hp, :], out_ps[s][0:HD, :], bc[:]
                    )
            # output projection for this q-chunk
            for t8 in range(SC // P):
                trow = q0 + t8 * P
                for co in range(2):
                    po = ps_op.tile([P, SC], F32, tag="psop")
                    for ec in range(E // P):
                        nc.tensor.matmul(
                            po[:], attnp[:, ec, t8 * P:(t8 + 1) * P],
                            woT_sb[:, ec, co * SC:(co + 1) * SC],
                            start=(ec == 0), stop=(ec == E // P - 1),
                        )
                    st = ost_pool.tile([P, SC], F32, tag="ost")
                    nc.vector.tensor_copy(st[:], po[:])
                    nc.sync.dma_start(
                        outp.ap()[trow:trow + P, co * SC:(co + 1) * SC], st[:]
                    )

    nc.compile()
    return nc


def _prep_inputs(hidden_states, qkv_w, qkv_b, o_w, o_b):
    hidden_states = np.asarray(hidden_states, dtype=np.float32)
    qkv_w = np.asarray(qkv_w, dtype=np.float32)
    qkv_b = np.asarray(qkv_b, dtype=np.float32)
    o_w = np.asarray(o_w, dtype=np.float32)

    msk = np.zeros((P, 4, SC), dtype=np.float32)
    j = np.arange(P)[:, None]
    q = np.arange(SC)[None, :]
    for i in range(4):
        msk[:, i, :] = ((P * i + j) <= q).astype(np.float32)

    in_maps = []
    for c in range(NCORES):
        b, g = c // 2, c % 2
        hsT = np.ascontiguousarray(hidden_states[b].T)
        qsel = qkv_w[E * g:E * g + E]
        ksel = qkv_w[C + E * g:C + E * g + E]
        vsel = qkv_w[2 * C + E * g:2 * C + E * g + E]
        wqkvT = np.ascontiguousarray(np.concatenate([qsel, ksel, vsel], 0).T)
        woT = np.ascontiguousarray(o_w[:, E * g:E * g + E].T)
        bq = qkv_b[E * g:E * g + E].reshape(4, P).T
        bk = qkv_b[C + E * g:C + E * g + E].reshape(4, P).T
        bv = qkv_b[2 * C + E * g:2 * C + E * g + E]
        bqkv = np.ascontiguousarray(np.concatenate([bq, bk], 1))
        vbias = np.ascontiguousarray(np.tile(bv[None, :], (P, 1)))
        in_maps.append({
            "hsT": hsT, "wqkvT": wqkvT, "woT": woT,
            "bqkv": bqkv, "vbias": vbias, "masks": msk,
        })
    return in_maps


def _get_nc():
    if "nc" not in _cache:
        _cache["nc"] = _build()
    return _cache["nc"]


def _run(in_maps, **kwargs):
    return run_bass_kernel_spmd(
        _get_nc(), in_maps, core_ids=list(range(NCORES)), **kwargs
    )


def kernel(hidden_states, qkv_w, qkv_b, o_w, o_b, **_):
    in_maps = _prep_inputs(hidden_states, qkv_w, qkv_b, o_w, o_b)
    res = _run(in_maps)
    o_b = np.asarray(o_b, dtype=np.float32)
    out = np.empty((B, T, C), dtype=np.float32)
    for b in range(B):
        out[b] = res.results[2 * b]["outp"] + res.results[2 * b + 1]["outp"] + o_b
    return out


# revision 10
# speedup vs baseline: 1.1063x; 1.1063x over previous
"""Causal self-attention kernel for 8 Trainium2 NeuronCores.

Problem: B=4, T=2048, C=1024, H=16 heads, HD=64.
  qkv = hs @ qkv_w.T + qkv_b ; per-head causal softmax attention ;
  out = attn @ o_w.T + o_b

Sharding (8 cores): core c handles batch b = c//2 and head-half g = c%2
(8 heads). Each core computes q/k/v for its heads from its batch's
hidden states, runs causal attention, and produces a partial output
projection over its 512 attention-output channels. The host sums the
two partials per batch and adds o_b.

On-device layout/dataflow (per core):
  - host pre-transposes hs and weights so every matmul contraction dim
    lands on SBUF partitions with contiguous DMA lines (no on-device
    transposes).
  - qT, kT stored [d, t] (d on partitions); v stored [t, d] natural,
    augmented with a ones-column so the PV matmul's row 64 accumulates
    the softmax denominator for free.
  - scores computed transposed [j, q] in PSUM; softmax has no
    max-subtraction (scores are ~N(0,1); exp is safe in fp32);
    causal mask applied multiplicatively on the exp'd tile only for
    diagonal-straddling j-chunks.
  - two heads share the PE array via tile_position row packing (K=64).
  - all matmuls in float32r (full PE rate, ~1.6e-4 relative rounding).
"""
import numpy as np
from contextlib import ExitStack

import concourse.bass as bass
from concourse import bacc
import concourse.tile as tile
import concourse.mybir as mybir
from concourse.bass_utils import run_bass_kernel_spmd

B, T, C = 4, 2048, 1024
H, HD = 16, 64
NCORES = 8
HPC = H // 2            # 8 heads per core
E = HPC * HD            # 512 local attn-out channels per core
P = 128
SC = 512                # q-chunk (matmul free dim)
NQC = T // SC           # 4 q-chunks
NJC = T // P            # 16 j-chunks
F32 = mybir.dt.float32
F32R = mybir.dt.float32r
Exp = mybir.ActivationFunctionType.Exp
SCALE = HD ** -0.5

_cache = {}


def _build():
    nc = bacc.Bacc("TRN2", target_bir_lowering=False, debug=False)
    hsT = nc.dram_tensor("hsT", [C, T], F32R, kind="ExternalInput")
    wqkvT = nc.dram_tensor("wqkvT", [C, 3 * E], F32R, kind="ExternalInput")
    woT = nc.dram_tensor("woT", [E, C], F32R, kind="ExternalInput")
    bqkv = nc.dram_tensor("bqkv", [P, 8], F32, kind="ExternalInput")
    vbias = nc.dram_tensor("vbias", [P, E], F32, kind="ExternalInput")
    masks = nc.dram_tensor("masks", [P, 4, SC], F32, kind="ExternalInput")
    outp = nc.dram_tensor("outp", [T, C], F32, kind="ExternalOutput")

    with tile.TileContext(nc) as tc, ExitStack() as ctx:
        const_pool = ctx.enter_context(tc.tile_pool(name="const", bufs=1))
        qk_pool = ctx.enter_context(tc.tile_pool(name="qk", bufs=1))

        bqkv_sb = const_pool.tile([P, 8], F32)
        vbias_sb = const_pool.tile([P, E], F32)
        masks_sb = const_pool.tile([P, 4, SC], F32)
        ones_sb = const_pool.tile([P, 1], F32)
        nc.sync.dma_start(bqkv_sb[:], bqkv.ap())
        nc.sync.dma_start(vbias_sb[:], vbias.ap())
        nc.sync.dma_start(masks_sb[:], masks.ap())
        nc.vector.memset(ones_sb[:], 1.0)

        qT = qk_pool.tile([P, 4, T], F32R)            # [d%128, d//128, t]
        kT = qk_pool.tile([P, 4, T], F32R)
        v_aug = qk_pool.tile([P, NJC, HPC, HD + 1], F32R)  # [t%128, jc, h, d|1]
        nc.vector.tensor_copy(
            v_aug[:, :, :, HD], ones_sb[:, 0, None, None].to_broadcast((P, NJC, HPC))
        )

        # one PSUM pool for everything: 4 banks for [128,512] tiles
        # (QKV accum / scores / o-proj share), 4 banks for PV accumulators
        ps_all = ctx.enter_context(tc.tile_pool(name="ps", bufs=4, space="PSUM"))

        # ---- Phase 1: fused QKV projection ----
        with tc.tile_pool(name="wq", bufs=1) as wq_pool, \
             tc.tile_pool(name="hst", bufs=3) as hst_pool:
            psq = ps_all
            wqkvT_sb = wq_pool.tile([P, C // P, 3 * E], F32R)
            wq_src = wqkvT.ap().rearrange("(co p) d -> p co d", p=P)
            for cc in range(C // P):
                nc.sync.dma_start(wqkvT_sb[:, cc], wq_src[:, cc])
            for t4 in range(NQC):
                hst = hst_pool.tile([P, C // P, SC], F32R, tag="hst")
                nc.sync.dma_start(
                    hst[:],
                    hsT.ap()[:, t4 * SC:(t4 + 1) * SC].rearrange(
                        "(co p) t -> p co t", p=P
                    ),
                )
                for qk in range(2):  # 0 -> qT, 1 -> kT
                    dstT = qT if qk == 0 else kT
                    for dc in range(4):
                        ps = psq.tile([P, SC], F32, tag="scps", name="psq")
                        w0 = qk * E + dc * P
                        for cc in range(C // P):
                            nc.tensor.matmul(
                                ps[:], wqkvT_sb[:, cc, w0:w0 + P], hst[:, cc, :],
                                start=(cc == 0), stop=(cc == C // P - 1),
                            )
                        nc.vector.tensor_add(
                            dstT[:, dc, t4 * SC:(t4 + 1) * SC], ps[:],
                            bqkv_sb[:, qk * 4 + dc, None].to_broadcast((P, SC)),
                        )
                for ts in range(4):  # v natural [t, d]
                    ps = psq.tile([P, E], F32, tag="scps", name="psv")
                    for cc in range(C // P):
                        nc.tensor.matmul(
                            ps[:], hst[:, cc, ts * P:(ts + 1) * P],
                            wqkvT_sb[:, cc, 2 * E:3 * E],
                            start=(cc == 0), stop=(cc == C // P - 1),
                        )
                    jc = t4 * 4 + ts
                    nc.vector.tensor_add(
                        v_aug[:, jc, :, 0:HD],
                        ps[:].rearrange("p (h d) -> p h d", d=HD),
                        vbias_sb[:].rearrange("p (h d) -> p h d", d=HD),
                    )

        # ---- Phase 2: attention + output projection ----
        wo_pool = ctx.enter_context(tc.tile_pool(name="wo", bufs=1))
        attnp_pool = ctx.enter_context(tc.tile_pool(name="attnp", bufs=2))
        exp_pool = ctx.enter_context(tc.tile_pool(name="expp", bufs=8))
        bc_pool = ctx.enter_context(tc.tile_pool(name="bcp", bufs=4))
        rc_pool = ctx.enter_context(tc.tile_pool(name="rcp", bufs=4))
        ost_pool = ctx.enter_context(tc.tile_pool(name="ost", bufs=3))
        ps_sc = ps_all
        ps_out = ps_all
        ps_op = ps_all

        woT_sb = wo_pool.tile([P, E // P, C], F32R)
        nc.sync.dma_start(woT_sb[:], woT.ap().rearrange("(ec p) co -> p ec co", p=P))

        for qc in range(NQC):
            attnp = attnp_pool.tile([P, E // P, SC], F32R, tag="attnp")
            q0 = qc * SC
            nj = 4 * (qc + 1)
            for hp in range(4):  # head pairs (2hp, 2hp+1)
                out_ps = [
                    ps_out.tile([HD + 1, SC], F32, tag="outps", name=f"outps{s}")
                    for s in range(2)
                ]
                pend = None  # deferred PV pair (software pipeline skew)

                def emit_pv(item):
                    jc, n0, exps = item
                    for s in range(2):
                        nc.tensor.matmul(
                            out_ps[s][:, n0:SC], v_aug[:, jc, 2 * hp + s, :],
                            exps[s][:, n0:SC],
                            start=(jc == 0), stop=(jc == nj - 1),
                        )

                for jc in range(nj):
                    di = jc - 4 * qc  # >= 0 on diagonal-straddling chunks
                    n0 = P * di if di >= 0 else 0
                    j0 = jc * P
                    scs, exps = [], []
                    for s in range(2):
                        sc_ps = ps_sc.tile([P, SC], F32, tag="scps", name="scps")
                        nc.tensor.matmul(
                            sc_ps[:, n0:SC],
                            kT[64 * s:64 * s + 64, hp, j0:j0 + P],
                            qT[64 * s:64 * s + 64, hp, q0 + n0:q0 + SC],
                            start=True, stop=True, tile_position=(64 * s, 0),
                        )
                        scs.append(sc_ps)
                    if pend is not None:
                        emit_pv(pend)
                    for s in range(2):
                        e = exp_pool.tile([P, SC], F32R, tag="exp")
                        nc.scalar.activation(
                            e[:, n0:SC], scs[s][:, n0:SC], Exp, scale=SCALE
                        )
                        if di >= 0:
                            nc.vector.tensor_mul(
                                e[:, n0:SC], e[:, n0:SC].bitcast(F32),
                                masks_sb[:, di, n0:SC],
                            )
                        exps.append(e)
                    pend = (jc, n0, exps)
                emit_pv(pend)
                # normalize by the ones-row sum and place into attnp
                for s in range(2):
                    srow = rc_pool.tile([1, SC], F32, tag="srow")
                    nc.vector.tensor_copy(srow[:], out_ps[s][HD:HD + 1, :])
                    bc = bc_pool.tile([64, SC], F32, tag="bc")
                    nc.gpsimd.partition_broadcast(bc[:], srow[:])
                    rc = bc_pool.tile([64, SC], F32, tag="rcb")
                    nc.vector.reciprocal_approx_fast(rc[:], bc[:])
                    nc.vector.tensor_mul(
                        attnp[64 * s:64 * s + 64, hp, :], out_ps[s][0:HD, :], rc[:]
                    )
            # output projection for this q-chunk
            for t8 in range(SC // P):
                trow = q0 + t8 * P
                for co in range(2):
                    po = ps_op.tile([P, SC], F32, tag="scps", name="psop")
                    for ec in range(E // P):
                        nc.tensor.matmul(
                            po[:], attnp[:, ec, t8 * P:(t8 + 1) * P],
                            woT_sb[:, ec, co * SC:(co + 1) * SC],
                            start=(ec == 0), stop=(ec == E // P - 1),
                        )
                    st = ost_pool.tile([P, SC], F32, tag="ost")
                    nc.vector.tensor_copy(st[:], po[:])
                    nc.sync.dma_start(
                        outp.ap()[trow:trow + P, co * SC:(co + 1) * SC], st[:]
                    )

    nc.compile()
    return nc


def _prep_inputs(hidden_states, qkv_w, qkv_b, o_w, o_b):
    hidden_states = np.asarray(hidden_states, dtype=np.float32)
    qkv_w = np.asarray(qkv_w, dtype=np.float32)
    qkv_b = np.asarray(qkv_b, dtype=np.float32)
    o_w = np.asarray(o_w, dtype=np.float32)

    msk = np.zeros((P, 4, SC), dtype=np.float32)
    j = np.arange(P)[:, None]
    q = np.arange(SC)[None, :]
    for i in range(4):
        msk[:, i, :] = ((P * i + j) <= q).astype(np.float32)

    in_maps = []
    for c in range(NCORES):
        b, g = c // 2, c % 2
        hsT = np.ascontiguousarray(hidden_states[b].T)
        qsel = qkv_w[E * g:E * g + E]
        ksel = qkv_w[C + E * g:C + E * g + E]
        vsel = qkv_w[2 * C + E * g:2 * C + E * g + E]
        wqkvT = np.ascontiguousarray(np.concatenate([qsel, ksel, vsel], 0).T)
        woT = np.ascontiguousarray(o_w[:, E * g:E * g + E].T)
        bq = qkv_b[E * g:E * g + E].reshape(4, P).T
        bk = qkv_b[C + E * g:C + E * g + E].reshape(4, P).T
        bv = qkv_b[2 * C + E * g:2 * C + E * g + E]
        bqkv = np.ascontiguousarray(np.concatenate([bq, bk], 1))
        vbias = np.ascontiguousarray(np.tile(bv[None, :], (P, 1)))
        in_maps.append({
            "hsT": hsT, "wqkvT": wqkvT, "woT": woT,
            "bqkv": bqkv, "vbias": vbias, "masks": msk,
        })
    return in_maps


def _get_nc():
    if "nc" not in _cache:
        _cache["nc"] = _build()
    return _cache["nc"]


def _run(in_maps, **kwargs):
    return run_bass_kernel_spmd(
        _get_nc(), in_maps, core_ids=list(range(NCORES)), **kwargs
    )


def kernel(hidden_states, qkv_w, qkv_b, o_w, o_b, **_):
    in_maps = _prep_inputs(hidden_states, qkv_w, qkv_b, o_w, o_b)
    res = _run(in_maps)
    o_b = np.asarray(o_b, dtype=np.float32)
    out = np.empty((B, T, C), dtype=np.float32)
    for b in range(B):
        out[b] = res.results[2 * b]["outp"] + res.results[2 * b + 1]["outp"] + o_b
    return out


# revision 11
# speedup vs baseline: 1.4439x; 1.3052x over previous
"""Causal self-attention kernel for 8 Trainium2 NeuronCores.

Problem: B=4, T=2048, C=1024, H=16 heads, HD=64.
  qkv = hs @ qkv_w.T + qkv_b ; per-head causal softmax attention ;
  out = attn @ o_w.T + o_b

Sharding (8 cores): core c handles batch b = c//2 and head-half g = c%2
(8 heads). Each core computes q/k/v for its heads from its batch's
hidden states, runs causal attention, and produces a partial output
projection over its 512 attention-output channels. The host sums the
two partials per batch and adds o_b.

On-device layout/dataflow (per core):
  - host pre-transposes hs and weights so every matmul contraction dim
    lands on SBUF partitions with contiguous DMA lines (no on-device
    transposes).
  - qT, kT stored [d, t] (d on partitions); v stored [t, d] natural,
    augmented with a ones-column so the PV matmul's row 64 accumulates
    the softmax denominator for free.
  - scores computed transposed [j, q] in PSUM; softmax has no
    max-subtraction (scores are ~N(0,1); exp is safe in fp32);
    causal mask applied multiplicatively on the exp'd tile only for
    diagonal-straddling j-chunks.
  - two heads share the PE array via tile_position row packing (K=64)
    and their score tiles sit in one 2-bank PSUM tile so a single
    activation / mask / copy instruction covers the pair (ACT and DVE
    instruction overheads dominate otherwise).
  - all matmuls in float32r (full PE rate, ~1.6e-4 relative rounding).
"""
import numpy as np
from contextlib import ExitStack

import concourse.bass as bass
from concourse import bacc
import concourse.tile as tile
import concourse.mybir as mybir
from concourse.bass_utils import run_bass_kernel_spmd

B, T, C = 4, 2048, 1024
H, HD = 16, 64
NCORES = 8
HPC = H // 2            # 8 heads per core
E = HPC * HD            # 512 local attn-out channels per core
P = 128
SC = 512                # q-chunk (matmul free dim)
NQC = T // SC           # 4 q-chunks
NJC = T // P            # 16 j-chunks
CC = C // P             # 8 contraction chunks
F32 = mybir.dt.float32
F32R = mybir.dt.float32r
Exp = mybir.ActivationFunctionType.Exp
SCALE = HD ** -0.5

_cache = {}


def _build():
    nc = bacc.Bacc("TRN2", target_bir_lowering=False, debug=False)
    hsT = nc.dram_tensor("hsT", [C, T], F32R, kind="ExternalInput")
    wqkvT = nc.dram_tensor("wqkvT", [C, 3 * E], F32R, kind="ExternalInput")
    woT = nc.dram_tensor("woT", [E, C], F32R, kind="ExternalInput")
    bqkv = nc.dram_tensor("bqkv", [P, 8], F32, kind="ExternalInput")
    vbias = nc.dram_tensor("vbias", [P, E], F32, kind="ExternalInput")
    masks = nc.dram_tensor("masks", [P, 4, SC], F32, kind="ExternalInput")
    outp = nc.dram_tensor("outp", [T, C], F32, kind="ExternalOutput")

    with tile.TileContext(nc) as tc, ExitStack() as ctx:
        const_pool = ctx.enter_context(tc.tile_pool(name="const", bufs=1))
        qk_pool = ctx.enter_context(tc.tile_pool(name="qk", bufs=1))

        bqkv_sb = const_pool.tile([P, 8], F32)
        vbias_sb = const_pool.tile([P, E], F32)
        masks_sb = const_pool.tile([P, 4, SC], F32)
        ones_sb = const_pool.tile([P, 1], F32)
        nc.sync.dma_start(bqkv_sb[:], bqkv.ap())
        nc.vector.memset(ones_sb[:], 1.0)

        qT = qk_pool.tile([P, 4, T], F32R)            # [d%128, d//128, t]
        kT = qk_pool.tile([P, 4, T], F32R)
        v_aug = qk_pool.tile([P, NJC, HPC, HD + 1], F32R)  # [t%128, jc, h, d|1]
        nc.vector.tensor_copy(
            v_aug[:, :, :, HD], ones_sb[:, 0, None, None].to_broadcast((P, NJC, HPC))
        )

        # PSUM: 2 x 2-bank rotating tiles (QKV accum pairs / score pairs /
        # o-proj pairs) + 4 x 1-bank PV accumulators = 8 banks.
        ps_all = ctx.enter_context(tc.tile_pool(name="ps", bufs=2, space="PSUM"))
        ps_out = ctx.enter_context(tc.tile_pool(name="pso", bufs=4, space="PSUM"))

        # ---- Phase 1: fused QKV projection ----
        with tc.tile_pool(name="wq", bufs=1) as wq_pool, \
             tc.tile_pool(name="hst", bufs=2) as hst_pool:
            wqkvT_sb = wq_pool.tile([P, CC, 3 * E], F32R)
            wq_src = wqkvT.ap().rearrange("(co p) d -> p co d", p=P)
            hst0 = hst_pool.tile([P, CC, SC], F32R, tag="hst", name="hst")
            nc.sync.dma_start(
                hst0[:], hsT.ap()[:, 0:SC].rearrange("(co p) t -> p co t", p=P)
            )
            for cc in range(CC):
                nc.sync.dma_start(wqkvT_sb[:, cc], wq_src[:, cc])
            # non-startup-critical loads, after the phase-1 gating DMAs
            nc.sync.dma_start(vbias_sb[:], vbias.ap())
            nc.sync.dma_start(masks_sb[:], masks.ap())

            for t4 in range(NQC):
                if t4 == 0:
                    hst = hst0
                else:
                    hst = hst_pool.tile([P, CC, SC], F32R, tag="hst", name="hst")
                    nc.sync.dma_start(
                        hst[:],
                        hsT.ap()[:, t4 * SC:(t4 + 1) * SC].rearrange(
                            "(co p) t -> p co t", p=P
                        ),
                    )
                for qk in range(2):  # 0 -> qT, 1 -> kT
                    dstT = qT if qk == 0 else kT
                    for dp in range(2):  # dc pairs (2*dp, 2*dp+1)
                        ps = ps_all.tile([P, 2, SC], F32, tag="ps2", name="psq")
                        for half in range(2):
                            w0 = qk * E + (2 * dp + half) * P
                            for cc in range(CC):
                                nc.tensor.matmul(
                                    ps[:, half], wqkvT_sb[:, cc, w0:w0 + P],
                                    hst[:, cc, :],
                                    start=(cc == 0), stop=(cc == CC - 1),
                                )
                        nc.vector.tensor_add(
                            dstT[:, 2 * dp:2 * dp + 2, t4 * SC:(t4 + 1) * SC],
                            ps[:],
                            bqkv_sb[:, qk * 4 + 2 * dp:qk * 4 + 2 * dp + 2, None]
                            .to_broadcast((P, 2, SC)),
                        )
                for tp in range(2):  # t-subchunk pairs for v natural [t, d]
                    ps = ps_all.tile([P, 2, SC], F32, tag="ps2", name="psv")
                    for half in range(2):
                        ts = 2 * tp + half
                        for cc in range(CC):
                            nc.tensor.matmul(
                                ps[:, half], hst[:, cc, ts * P:(ts + 1) * P],
                                wqkvT_sb[:, cc, 2 * E:3 * E],
                                start=(cc == 0), stop=(cc == CC - 1),
                            )
                    jc = t4 * 4 + 2 * tp
                    nc.vector.tensor_add(
                        v_aug[:, jc:jc + 2, :, 0:HD],
                        ps[:].rearrange("p two (h d) -> p two h d", d=HD),
                        vbias_sb[:, None].rearrange("p two (h d) -> p two h d", d=HD)
                        .to_broadcast((P, 2, HPC, HD)),
                    )

        # ---- Phase 2: attention + output projection ----
        wo_pool = ctx.enter_context(tc.tile_pool(name="wo", bufs=1))
        attnp_pool = ctx.enter_context(tc.tile_pool(name="attnp", bufs=2))
        exp_pool = ctx.enter_context(tc.tile_pool(name="expp", bufs=4))
        bc_pool = ctx.enter_context(tc.tile_pool(name="bcp", bufs=4))
        rc_pool = ctx.enter_context(tc.tile_pool(name="rcp", bufs=4))
        ost_pool = ctx.enter_context(tc.tile_pool(name="ost", bufs=3))

        woT_sb = wo_pool.tile([P, E // P, C], F32R)
        nc.sync.dma_start(woT_sb[:], woT.ap().rearrange("(ec p) co -> p ec co", p=P))

        for qc in range(NQC):
            attnp = attnp_pool.tile([P, E // P, SC], F32R, tag="attnp")
            q0 = qc * SC
            nj = 4 * (qc + 1)
            for hp in range(4):  # head pairs (2hp, 2hp+1)
                out_ps = [
                    ps_out.tile([HD + 1, SC], F32, tag="outps", name=f"outps{s}")
                    for s in range(2)
                ]
                pend = None  # deferred PV pair (software pipeline skew)

                def emit_pv(item):
                    jc, n0, e = item
                    for s in range(2):
                        nc.tensor.matmul(
                            out_ps[s][:, n0:SC], v_aug[:, jc, 2 * hp + s, :],
                            e[:, s, n0:SC],
                            start=(jc == 0), stop=(jc == nj - 1),
                        )

                for jc in range(nj):
                    di = jc - 4 * qc  # >= 0 on diagonal-straddling chunks
                    n0 = P * di if di >= 0 else 0
                    j0 = jc * P
                    sc_ps = ps_all.tile([P, 2, SC], F32, tag="ps2", name="scps")
                    for s in range(2):
                        nc.tensor.matmul(
                            sc_ps[:, s, n0:SC],
                            kT[64 * s:64 * s + 64, hp, j0:j0 + P],
                            qT[64 * s:64 * s + 64, hp, q0 + n0:q0 + SC],
                            start=True, stop=True, tile_position=(64 * s, 0),
                        )
                    if pend is not None:
                        emit_pv(pend)
                    e = exp_pool.tile([P, 2, SC], F32R, tag="exp")
                    nc.scalar.activation(
                        e[:, :, n0:SC], sc_ps[:, :, n0:SC], Exp, scale=SCALE
                    )
                    if di >= 0:
                        nc.vector.tensor_mul(
                            e[:, :, n0:SC], e[:, :, n0:SC].bitcast(F32),
                            masks_sb[:, di, None, n0:SC].to_broadcast(
                                (P, 2, SC - n0)
                            ),
                        )
                    pend = (jc, n0, e)
                emit_pv(pend)
                # normalize by the ones-row sum and place into attnp
                for s in range(2):
                    srow = rc_pool.tile([1, SC], F32, tag="srow")
                    nc.vector.tensor_copy(srow[:], out_ps[s][HD:HD + 1, :])
                    bc = bc_pool.tile([64, SC], F32, tag="bc")
                    nc.gpsimd.partition_broadcast(bc[:], srow[:])
                    rc = bc_pool.tile([64, SC], F32, tag="rcb")
                    nc.vector.reciprocal_approx_fast(rc[:], bc[:])
                    nc.vector.tensor_mul(
                        attnp[64 * s:64 * s + 64, hp, :], out_ps[s][0:HD, :], rc[:]
                    )
            # output projection for this q-chunk (both co halves in one
            # 2-bank psum tile -> one copy, one contiguous-row DMA)
            for t8 in range(SC // P):
                trow = q0 + t8 * P
                po = ps_all.tile([P, 2, SC], F32, tag="ps2", name="psop")
                for co in range(2):
                    for ec in range(E // P):
                        nc.tensor.matmul(
                            po[:, co], attnp[:, ec, t8 * P:(t8 + 1) * P],
                            woT_sb[:, ec, co * SC:(co + 1) * SC],
                            start=(ec == 0), stop=(ec == E // P - 1),
                        )
                st = ost_pool.tile([P, 2, SC], F32, tag="ost")
                nc.vector.tensor_copy(st[:], po[:])
                nc.sync.dma_start(outp.ap()[trow:trow + P, :], st[:])

    nc.compile()
    return nc


def _prep_inputs(hidden_states, qkv_w, qkv_b, o_w, o_b):
    hidden_states = np.asarray(hidden_states, dtype=np.float32)
    qkv_w = np.asarray(qkv_w, dtype=np.float32)
    qkv_b = np.asarray(qkv_b, dtype=np.float32)
    o_w = np.asarray(o_w, dtype=np.float32)

    msk = np.zeros((P, 4, SC), dtype=np.float32)
    j = np.arange(P)[:, None]
    q = np.arange(SC)[None, :]
    for i in range(4):
        msk[:, i, :] = ((P * i + j) <= q).astype(np.float32)

    in_maps = []
    for c in range(NCORES):
        b, g = c // 2, c % 2
        hsT = np.ascontiguousarray(hidden_states[b].T)
        qsel = qkv_w[E * g:E * g + E]
        ksel = qkv_w[C + E * g:C + E * g + E]
        vsel = qkv_w[2 * C + E * g:2 * C + E * g + E]
        wqkvT = np.ascontiguousarray(np.concatenate([qsel, ksel, vsel], 0).T)
        woT = np.ascontiguousarray(o_w[:, E * g:E * g + E].T)
        bq = qkv_b[E * g:E * g + E].reshape(4, P).T
        bk = qkv_b[C + E * g:C + E * g + E].reshape(4, P).T
        bv = qkv_b[2 * C + E * g:2 * C + E * g + E]
        bqkv = np.ascontiguousarray(np.concatenate([bq, bk], 1))
        vbias = np.ascontiguousarray(np.tile(bv[None, :], (P, 1)))
        in_maps.append({
            "hsT": hsT, "wqkvT": wqkvT, "woT": woT,
            "bqkv": bqkv, "vbias": vbias, "masks": msk,
        })
    return in_maps


def _get_nc():
    if "nc" not in _cache:
        _cache["nc"] = _build()
    return _cache["nc"]


def _run(in_maps, **kwargs):
    return run_bass_kernel_spmd(
        _get_nc(), in_maps, core_ids=list(range(NCORES)), **kwargs
    )


def kernel(hidden_states, qkv_w, qkv_b, o_w, o_b, **_):
    in_maps = _prep_inputs(hidden_states, qkv_w, qkv_b, o_w, o_b)
    res = _run(in_maps)
    o_b = np.asarray(o_b, dtype=np.float32)
    out = np.empty((B, T, C), dtype=np.float32)
    for b in range(B):
        out[b] = res.results[2 * b]["outp"] + res.results[2 * b + 1]["outp"] + o_b
    return out
